# revision 14
# baseline (speedup 1.0000x reference)
"""Multi-head causal attention with RoPE on 8 TRN2 NeuronCores.

Sharding: data-parallel over batch (B=2) x tensor-parallel over head groups
(16 heads -> 4 groups of 4). Core c handles batch c//4, heads [4*(c%4), 4*(c%4)+4).
Each core computes its partial output projection; the host sums the 4 partial
outputs per batch (the "all-reduce after w_o").

Per-core device pipeline (transpose-free attention):
  A) load x^T, W slices (host-pretransposed), RoPE tables
  B) QKV projection with fp32r matmuls: Q^T,K^T in [d, s] layout; V in [s, d]
     bf16 with a ones-column appended per head (for softmax row sums)
  C) RoPE on Q^T/K^T via partition-pair-swap DMA + 3 DVE ops
  D) per head-pair: scores computed K-MAJOR as ST[k, q] blocks (row-packed
     2 heads, K=64), exp on ScalarE straight into PV-ready bf16 tiles,
     causal zeroing via gpsimd affine_select, PV with M=65 (ones row
     accumulates the softmax denominators l[q]), then normalization by
     1/l via a K=1 broadcast matmul + DVE multiply into attnT.
  E) output projection back to [s, o] layout, DMA out
"""

import math
import numpy as np

import concourse.bass as bass
import concourse.tile as tile
from concourse import bacc, mybir
from concourse.bass_utils import run_bass_kernel_spmd

B, S, D, H, DK = 2, 2048, 1024, 16, 64
NCORES = 8
GROUPS = 4
ROPE_THETA = 10000.0

F32 = mybir.dt.float32
F32R = mybir.dt.float32r
BF16 = mybir.dt.bfloat16
EXP = mybir.ActivationFunctionType.Exp
AX = mybir.AxisListType.X
GE = mybir.AluOpType.is_ge

_PROGRAM = None
LAST_RESULTS = None  # BassKernelResults of the last kernel() call (for test.py)


def _emit(tc, t_xT, t_wqkT, t_wvT, t_woT, t_cos, t_ssin, t_perm, t_out):
    nc = tc.nc
    xT = t_xT.ap()          # [1024, 2048] f32  (x[b]^T)
    wqkT = t_wqkT.ap()      # [1024, 512] f32   (cols: Q h0 h1 h2 h3 | K h0..h3)
    wvT = t_wvT.ap()        # [1024, 256] f32
    woT = t_woT.ap()        # [256, 1024] f32
    cosd = t_cos.ap()       # [128, 2048] f32  (2-head stacked rope cos, [d, s])
    ssin = t_ssin.ap()      # [128, 2048] f32  (signed sin, [d, s])
    perm = t_perm.ap()      # [128, 128] f32 pair-swap permutation
    out = t_out.ap()        # [2048, 1024] f32

    with tc.tile_pool(name="persist", bufs=1) as pers:
        qkT = [pers.tile([128, S], F32R, tag=f"qkT{t}", name=f"qkT{t}") for t in range(4)]
        #   qkT[0]=Q pair0 (heads 0,1), qkT[1]=Q pair1, qkT[2]=K pair0, qkT[3]=K pair1
        # v2[st]: [128 s, 4*65] bf16: per head h: cols 65h..65h+63 = V_h, col 65h+64 = 1.0
        v2_sb = [pers.tile([128, 4 * 65], BF16, tag=f"v{st}", name=f"v{st}") for st in range(16)]
        attnT = [pers.tile([128, S], F32R, tag=f"attnT{p}", name=f"attnT{p}") for p in range(2)]
        woT_sb = [pers.tile([128, 1024], F32R, tag=f"woT{i}", name=f"woT{i}") for i in range(2)]
        cos_sb = pers.tile([128, S], F32, tag="cos")
        ssin_sb = pers.tile([128, S], F32, tag="ssin")
        qkB = [pers.tile([128, S], BF16, tag=f"qkB{t}", name=f"qkB{t}") for t in range(4)]

        perm_sb = pers.tile([128, 128], F32R, tag="perm")
        nc.sync.dma_start(out=cos_sb, in_=cosd)
        nc.sync.dma_start(out=ssin_sb, in_=ssin)
        nc.sync.dma_start(out=perm_sb, in_=perm.bitcast(F32R))
        for i in range(2):
            nc.sync.dma_start(out=woT_sb[i], in_=woT[128 * i:128 * (i + 1), :].bitcast(F32R))

        # ---- Phase B: QKV projection ----
        with tc.tile_pool(name="inw", bufs=1) as inw, \
             tc.tile_pool(name="psB", bufs=2, space="PSUM") as psB:
            xT_sb = [inw.tile([128, S], F32R, tag=f"xT{i}", name=f"xT{i}") for i in range(8)]
            wqk_sb = [inw.tile([128, 512], F32R, tag=f"wqk{i}", name=f"wqk{i}") for i in range(8)]
            wv_sb = [inw.tile([128, 256], F32R, tag=f"wv{i}", name=f"wv{i}") for i in range(8)]
            for r in range(4):
                for i in range(8):
                    nc.sync.dma_start(
                        out=wqk_sb[i][:, 128 * r:128 * (r + 1)],
                        in_=wqkT[128 * i:128 * (i + 1), 128 * r:128 * (r + 1)].bitcast(F32R))
                    nc.sync.dma_start(
                        out=xT_sb[i][:, 512 * r:512 * (r + 1)],
                        in_=xT[128 * i:128 * (i + 1), 512 * r:512 * (r + 1)].bitcast(F32R))
                if r == 1:
                    for i in range(8):
                        nc.sync.dma_start(out=wv_sb[i], in_=wvT[128 * i:128 * (i + 1), :].bitcast(F32R))

            # Q^T / K^T proj + fused RoPE -> qkB (bf16); pair-0 tiles first
            with tc.tile_pool(name="ropeP", bufs=3) as rpp, \
                 tc.tile_pool(name="psSW", bufs=2, space="PSUM") as psSW:
                for ot in (0, 2, 1, 3):
                    for st in range(4):
                        csl = slice(512 * st, 512 * (st + 1))
                        ps = psB.tile([128, 512], F32, tag="psQK")
                        for it in range(8):
                            nc.tensor.matmul(
                                ps,
                                wqk_sb[it][:, 128 * ot:128 * (ot + 1)],
                                xT_sb[it][:, 512 * st:512 * (st + 1)],
                                start=(it == 0), stop=(it == 7),
                            )
                        nc.scalar.copy(out=qkT[ot][:, csl], in_=ps)
                        # rope: swp = perm @ qkT chunk (pair swap via PE)
                        sw_ps = psSW.tile([128, 512], F32, tag="sw")
                        nc.tensor.matmul(sw_ps, perm_sb, qkT[ot][:, csl],
                                         start=True, stop=True)
                        t1 = rpp.tile([128, 512], F32, tag="t1")
                        nc.vector.tensor_mul(out=t1, in0=sw_ps, in1=ssin_sb[:, csl])
                        t2 = rpp.tile([128, 512], F32, tag="t2")
                        nc.vector.tensor_mul(out=t2, in0=ps, in1=cos_sb[:, csl])
                        nc.vector.tensor_add(out=qkB[ot][:, csl], in0=t2, in1=t1)
            # V : out [s-tile 128, 256] -> v2 (strided dest, ones cols pre-set)
            for st in range(16):
                nc.vector.memset(v2_sb[st], 1.0)
                ps = psB.tile([128, 256], F32, tag="psV")
                for it in range(8):
                    nc.tensor.matmul(
                        ps,
                        xT_sb[it][:, 128 * st:128 * (st + 1)],
                        wv_sb[it],
                        start=(it == 0), stop=(it == 7),
                    )
                v2_view = v2_sb[st].rearrange("p (h c) -> p h c", c=65)[:, :, 0:64]
                ps_view = ps.rearrange("p (h c) -> p h c", c=64)
                nc.vector.tensor_copy(out=v2_view, in_=ps_view)

        # ---- Phase D: attention per head-pair, K-major (transpose-free) ----
        with tc.tile_pool(name="ptp", bufs=12) as ptp, \
             tc.tile_pool(name="nrm", bufs=6) as nrm, \
             tc.tile_pool(name="psS", bufs=2, space="PSUM") as psS, \
             tc.tile_pool(name="psV", bufs=2, space="PSUM") as psV:
            for p in range(2):
                Q, K = qkB[p], qkB[2 + p]
                hA, hB = 2 * p, 2 * p + 1
                for si in range(4):
                    qsl = slice(512 * si, 512 * (si + 1))
                    nkb = 4 * (si + 1)
                    PT = []
                    for kb in range(nkb):
                        ksl = slice(128 * kb, 128 * (kb + 1))
                        # paired A|B scores psum [128, 1024] (2 banks)
                        st2 = psS.tile([128, 1024], F32, tag="st2")
                        nc.tensor.matmul(st2[:, 0:512], K[0:64, ksl], Q[0:64, qsl],
                                         start=True, stop=True, tile_position=(0, 0))
                        nc.tensor.matmul(st2[:, 512:1024], K[64:128, ksl], Q[64:128, qsl],
                                         start=True, stop=True, tile_position=(64, 0))
                        pt2 = ptp.tile([128, 1024], BF16, tag="pt2")
                        c0 = 128 * (kb - 4 * si)  # first needed col (boundary tiles)
                        if c0 <= 0:
                            nc.scalar.activation(out=pt2, in_=st2, func=EXP,
                                                 scale=1.0 / math.sqrt(DK))
                        else:
                            st2v = st2.rearrange("p (h c) -> p h c", c=512)
                            pt2v = pt2.rearrange("p (h c) -> p h c", c=512)
                            nc.gpsimd.memset(pt2v[:, :, 0:c0], 0.0)
                            nc.scalar.activation(out=pt2v[:, :, c0:512],
                                                 in_=st2v[:, :, c0:512], func=EXP,
                                                 scale=1.0 / math.sqrt(DK))
                        if kb >= 4 * si:
                            # self-diagonal 128-col block: keep iff col >= part
                            pt2v = pt2.rearrange("p (h c) -> p h c", c=512)
                            nc.gpsimd.affine_select(
                                out=pt2v[:, :, c0:c0 + 128],
                                in_=pt2v[:, :, c0:c0 + 128],
                                pattern=[[0, 2], [1, 128]],
                                compare_op=GE, fill=0.0, base=0,
                                channel_multiplier=-1)
                        PT.append(pt2)
                        if kb == 0:
                            oA = psV.tile([65, 512], F32, tag="oA")
                            oB = psV.tile([65, 512], F32, tag="oB")
                        first, last = kb == 0, kb == nkb - 1
                        nc.tensor.matmul(oA, v2_sb[kb][:, 65 * hA:65 * hA + 65],
                                         pt2[:, 0:512], start=first, stop=last)
                        nc.tensor.matmul(oB, v2_sb[kb][:, 65 * hB:65 * hB + 65],
                                         pt2[:, 512:1024], start=first, stop=last)
                    # normalize: r = 1/l (row 64), broadcast rows via DMA
                    for o_ps, half in ((oA, 0), (oB, 1)):
                        ltmp = nrm.tile([1, 512], F32, tag="ltmp")
                        nc.vector.tensor_copy(out=ltmp, in_=o_ps[64:65, :])
                        rrow = nrm.tile([1, 512], F32, tag="rrow")
                        nc.vector.reciprocal_approx_fast(out=rrow, in_=ltmp)
                        rbc = nrm.tile([64, 512], F32, tag="rbc")
                        nc.gpsimd.partition_broadcast(rbc, rrow)
                        nc.vector.tensor_mul(
                            out=attnT[p][64 * half:64 * (half + 1), qsl],
                            in0=o_ps[0:64, :], in1=rbc)

        # ---- Phase E: output projection ----
        with tc.tile_pool(name="psE", bufs=4, space="PSUM") as psE, \
             tc.tile_pool(name="outp", bufs=3) as op:
            for st in range(16):
                ob = op.tile([128, 1024], F32, tag="ob")
                for oc in range(2):
                    pe = psE.tile([128, 512], F32, tag="pe")
                    nc.tensor.matmul(
                        pe,
                        attnT[0][:, 128 * st:128 * (st + 1)],
                        woT_sb[0][:, 512 * oc:512 * (oc + 1)],
                        start=True, stop=False)
                    nc.tensor.matmul(
                        pe,
                        attnT[1][:, 128 * st:128 * (st + 1)],
                        woT_sb[1][:, 512 * oc:512 * (oc + 1)],
                        start=False, stop=True)
                    if oc == 0:
                        nc.vector.tensor_copy(out=ob[:, 0:512], in_=pe)
                    else:
                        nc.scalar.copy(out=ob[:, 512:1024], in_=pe)
                nc.sync.dma_start(out=out[128 * st:128 * (st + 1), 0:512], in_=ob[:, 0:512])
                nc.sync.dma_start(out=out[128 * st:128 * (st + 1), 512:1024], in_=ob[:, 512:1024])


def _build_program():
    nc = bacc.Bacc("TRN2", debug=False, enable_asserts=False,
                   target_bir_lowering=False, num_devices=NCORES)
    t_xT = nc.dram_tensor("xT", [D, S], F32, kind="ExternalInput")
    t_wqkT = nc.dram_tensor("wqkT", [D, 512], F32, kind="ExternalInput")
    t_wvT = nc.dram_tensor("wvT", [D, 256], F32, kind="ExternalInput")
    t_woT = nc.dram_tensor("woT", [256, D], F32, kind="ExternalInput")
    t_cos = nc.dram_tensor("cosd", [128, S], F32, kind="ExternalInput")
    t_ssin = nc.dram_tensor("ssin", [128, S], F32, kind="ExternalInput")
    t_perm = nc.dram_tensor("perm", [128, 128], F32, kind="ExternalInput")
    t_out = nc.dram_tensor("out", [S, D], F32, kind="ExternalOutput")
    with tile.TileContext(nc) as tc:
        _emit(tc, t_xT, t_wqkT, t_wvT, t_woT, t_cos, t_ssin, t_perm, t_out)
    nc.compile()
    return nc


def _rope_tables():
    # [128, S] tables for a 2-head stacked [d, s] block (pattern repeats per 64)
    i = np.arange(0, DK, 2, dtype=np.float64) / DK
    inv_freq = ROPE_THETA ** i                       # [32]
    ang = np.arange(S, dtype=np.float64)[None, :] / inv_freq[:, None]  # [32, S]
    cos64 = np.repeat(np.cos(ang), 2, axis=0)        # [64, S]
    sin = np.sin(ang)
    ssin64 = np.empty((DK, S), dtype=np.float64)
    ssin64[0::2] = -sin
    ssin64[1::2] = sin
    cos128 = np.tile(cos64, (2, 1)).astype(np.float32)
    ssin128 = np.tile(ssin64, (2, 1)).astype(np.float32)
    return np.ascontiguousarray(cos128), np.ascontiguousarray(ssin128)


def kernel(x, W_qkv, W_o):
    global _PROGRAM, LAST_RESULTS
    x = np.asarray(x, dtype=np.float32)
    W_qkv = np.asarray(W_qkv, dtype=np.float32)
    W_o = np.asarray(W_o, dtype=np.float32)

    if _PROGRAM is None:
        _PROGRAM = _build_program()
    nc = _PROGRAM

    cos128, ssin128 = _rope_tables()
    permM = np.zeros((128, 128), dtype=np.float32)
    idx = np.arange(128)
    permM[idx, idx ^ 1] = 1.0  # lhsT[K=d, M=d']: out[d'] = sum_d perm[d, d'] q[d] = q[d'^1]

    in_maps = []
    for c in range(NCORES):
        b, g = c // 4, c % 4
        rq = W_qkv[256 * g:256 * (g + 1)]
        rk = W_qkv[D + 256 * g:D + 256 * (g + 1)]
        rv = W_qkv[2 * D + 256 * g:2 * D + 256 * (g + 1)]
        in_maps.append({
            "xT": np.ascontiguousarray(x[b].T),
            "wqkT": np.ascontiguousarray(np.concatenate([rq, rk], 0).T),
            "wvT": np.ascontiguousarray(rv.T),
            "woT": np.ascontiguousarray(W_o[:, 256 * g:256 * (g + 1)].T),
            "cosd": cos128,
            "ssin": ssin128,
            "perm": permM,
        })

    res = run_bass_kernel_spmd(nc, in_maps, core_ids=list(range(NCORES)))
    LAST_RESULTS = res

    out = np.empty((B, S, D), dtype=np.float32)
    for b in range(B):
        acc = np.zeros((S, D), dtype=np.float64)
        for g in range(GROUPS):
            acc += res.results[4 * b + g]["out"]
        out[b] = acc.astype(np.float32)
    return out


# revision 15
# speedup vs baseline: 1.0407x; 1.0407x over previous
"""Multi-head causal attention with RoPE on 8 TRN2 NeuronCores.

Sharding: data-parallel over batch (B=2) x tensor-parallel over head groups
(16 heads -> 4 groups of 4). Core c handles batch c//4, heads [4*(c%4), 4*(c%4)+4).
Each core computes its partial output projection; the host sums the 4 partial
outputs per batch (the "all-reduce after w_o").

Per-core device pipeline (transpose-free attention):
  A) load x^T, W slices (host-pretransposed), RoPE tables
  B) QKV projection with fp32r matmuls: Q^T,K^T in [d, s] layout; V in [s, d]
     bf16 with a ones-column appended per head (for softmax row sums)
  C) RoPE on Q^T/K^T via partition-pair-swap DMA + 3 DVE ops
  D) per head-pair: scores computed K-MAJOR as ST[k, q] blocks (row-packed
     2 heads, K=64), exp on ScalarE straight into PV-ready bf16 tiles,
     causal zeroing via gpsimd affine_select, PV with M=65 (ones row
     accumulates the softmax denominators l[q]), then normalization by
     1/l via a K=1 broadcast matmul + DVE multiply into attnT.
  E) output projection back to [s, o] layout, DMA out
"""

import math
import numpy as np

import concourse.bass as bass
import concourse.tile as tile
from concourse import bacc, mybir
from concourse.bass_utils import run_bass_kernel_spmd

B, S, D, H, DK = 2, 2048, 1024, 16, 64
NCORES = 8
GROUPS = 4
ROPE_THETA = 10000.0

F32 = mybir.dt.float32
F32R = mybir.dt.float32r
BF16 = mybir.dt.bfloat16
EXP = mybir.ActivationFunctionType.Exp
AX = mybir.AxisListType.X
GE = mybir.AluOpType.is_ge

_PROGRAM = None
LAST_RESULTS = None  # BassKernelResults of the last kernel() call (for test.py)


def _emit(tc, t_xT, t_wqkT, t_wvT, t_woT, t_cos, t_ssin, t_perm, t_out):
    nc = tc.nc
    xT = t_xT.ap()          # [1024, 2048] f32  (x[b]^T)
    wqkT = t_wqkT.ap()      # [1024, 512] f32   (cols: Q h0 h1 h2 h3 | K h0..h3)
    wvT = t_wvT.ap()        # [1024, 256] f32
    woT = t_woT.ap()        # [256, 1024] f32
    cosd = t_cos.ap()       # [128, 2048] f32  (2-head stacked rope cos, [d, s])
    ssin = t_ssin.ap()      # [128, 2048] f32  (signed sin, [d, s])
    perm = t_perm.ap()      # [128, 128] f32 pair-swap permutation
    out = t_out.ap()        # [2048, 1024] f32

    with tc.tile_pool(name="persist", bufs=1) as pers:
        qkT = [pers.tile([128, S], F32R, tag=f"qkT{t}", name=f"qkT{t}") for t in range(4)]
        #   qkT[0]=Q pair0 (heads 0,1), qkT[1]=Q pair1, qkT[2]=K pair0, qkT[3]=K pair1
        # v2[st]: [128 s, 4*65] bf16: per head h: cols 65h..65h+63 = V_h, col 65h+64 = 1.0
        v2_sb = [pers.tile([128, 4 * 65], BF16, tag=f"v{st}", name=f"v{st}") for st in range(16)]
        attnT = [pers.tile([128, S], F32R, tag=f"attnT{p}", name=f"attnT{p}") for p in range(2)]
        woT_sb = [pers.tile([128, 1024], F32R, tag=f"woT{i}", name=f"woT{i}") for i in range(2)]
        cos_sb = pers.tile([128, S], F32, tag="cos")
        ssin_sb = pers.tile([128, S], F32, tag="ssin")
        qkB = [pers.tile([128, S], BF16, tag=f"qkB{t}", name=f"qkB{t}") for t in range(4)]

        perm_sb = pers.tile([128, 128], F32R, tag="perm")
        nc.sync.dma_start(out=cos_sb, in_=cosd)
        nc.sync.dma_start(out=ssin_sb, in_=ssin)
        nc.sync.dma_start(out=perm_sb, in_=perm.bitcast(F32R))
        for i in range(2):
            nc.sync.dma_start(out=woT_sb[i], in_=woT[128 * i:128 * (i + 1), :].bitcast(F32R))

        # ---- Phase B: QKV projection ----
        with tc.tile_pool(name="inw", bufs=1) as inw, \
             tc.tile_pool(name="psB", bufs=2, space="PSUM") as psB:
            xT_sb = [inw.tile([128, S], F32R, tag=f"xT{i}", name=f"xT{i}") for i in range(8)]
            wqk_sb = [inw.tile([128, 512], F32R, tag=f"wqk{i}", name=f"wqk{i}") for i in range(8)]
            wv_sb = [inw.tile([128, 256], F32R, tag=f"wv{i}", name=f"wv{i}") for i in range(8)]
            for r in range(4):
                for i in range(8):
                    nc.sync.dma_start(
                        out=wqk_sb[i][:, 128 * r:128 * (r + 1)],
                        in_=wqkT[128 * i:128 * (i + 1), 128 * r:128 * (r + 1)].bitcast(F32R))
                    nc.sync.dma_start(
                        out=xT_sb[i][:, 512 * r:512 * (r + 1)],
                        in_=xT[128 * i:128 * (i + 1), 512 * r:512 * (r + 1)].bitcast(F32R))
                if r == 1:
                    for i in range(8):
                        nc.sync.dma_start(out=wv_sb[i], in_=wvT[128 * i:128 * (i + 1), :].bitcast(F32R))

            # Q^T / K^T proj + fused RoPE -> qkB (bf16); pair-0 tiles first
            with tc.tile_pool(name="ropeP", bufs=3) as rpp, \
                 tc.tile_pool(name="psSW", bufs=2, space="PSUM") as psSW:
                for ot in (0, 2, 1, 3):
                    for st in range(4):
                        csl = slice(512 * st, 512 * (st + 1))
                        ps = psB.tile([128, 512], F32, tag="psQK")
                        for it in range(8):
                            nc.tensor.matmul(
                                ps,
                                wqk_sb[it][:, 128 * ot:128 * (ot + 1)],
                                xT_sb[it][:, 512 * st:512 * (st + 1)],
                                start=(it == 0), stop=(it == 7),
                            )
                        nc.scalar.copy(out=qkT[ot][:, csl], in_=ps)
                        # rope: swp = perm @ qkT chunk (pair swap via PE)
                        sw_ps = psSW.tile([128, 512], F32, tag="sw")
                        nc.tensor.matmul(sw_ps, perm_sb, qkT[ot][:, csl],
                                         start=True, stop=True)
                        t1 = rpp.tile([128, 512], F32, tag="t1")
                        nc.vector.tensor_mul(out=t1, in0=sw_ps, in1=ssin_sb[:, csl])
                        t2 = rpp.tile([128, 512], F32, tag="t2")
                        nc.vector.tensor_mul(out=t2, in0=ps, in1=cos_sb[:, csl])
                        nc.vector.tensor_add(out=qkB[ot][:, csl], in0=t2, in1=t1)
            # V : out [s-tile 128, 256] -> v2 (strided dest, ones cols pre-set)
            for st in range(16):
                nc.vector.memset(v2_sb[st], 1.0)
                ps = psB.tile([128, 256], F32, tag="psV")
                for it in range(8):
                    nc.tensor.matmul(
                        ps,
                        xT_sb[it][:, 128 * st:128 * (st + 1)],
                        wv_sb[it],
                        start=(it == 0), stop=(it == 7),
                    )
                v2_view = v2_sb[st].rearrange("p (h c) -> p h c", c=65)[:, :, 0:64]
                ps_view = ps.rearrange("p (h c) -> p h c", c=64)
                nc.vector.tensor_copy(out=v2_view, in_=ps_view)

        # ---- Phase D: attention per head-pair, K-major (transpose-free) ----
        with tc.tile_pool(name="ptp", bufs=12) as ptp, \
             tc.tile_pool(name="nrm", bufs=6) as nrm, \
             tc.tile_pool(name="psS", bufs=2, space="PSUM") as psS, \
             tc.tile_pool(name="psV", bufs=2, space="PSUM") as psV:
            for p in range(2):
                Q, K = qkB[p], qkB[2 + p]
                hA, hB = 2 * p, 2 * p + 1
                for si in range(4):
                    qsl = slice(512 * si, 512 * (si + 1))
                    nkb = 4 * (si + 1)
                    PT = []
                    for kb in range(nkb):
                        ksl = slice(128 * kb, 128 * (kb + 1))
                        # paired A|B scores psum [128, 1024] (2 banks)
                        st2 = psS.tile([128, 1024], F32, tag="st2")
                        nc.tensor.matmul(st2[:, 0:512], K[0:64, ksl], Q[0:64, qsl],
                                         start=True, stop=True, tile_position=(0, 0))
                        nc.tensor.matmul(st2[:, 512:1024], K[64:128, ksl], Q[64:128, qsl],
                                         start=True, stop=True, tile_position=(64, 0))
                        pt2 = ptp.tile([128, 1024], BF16, tag="pt2")
                        c0 = 128 * (kb - 4 * si)  # first needed col (boundary tiles)
                        if c0 <= 0:
                            nc.scalar.activation(out=pt2, in_=st2, func=EXP,
                                                 scale=1.0 / math.sqrt(DK))
                        else:
                            st2v = st2.rearrange("p (h c) -> p h c", c=512)
                            pt2v = pt2.rearrange("p (h c) -> p h c", c=512)
                            nc.gpsimd.memset(pt2v[:, :, 0:c0], 0.0)
                            nc.scalar.activation(out=pt2v[:, :, c0:512],
                                                 in_=st2v[:, :, c0:512], func=EXP,
                                                 scale=1.0 / math.sqrt(DK))
                        if kb >= 4 * si:
                            # self-diagonal 128-col block: keep iff col >= part
                            pt2v = pt2.rearrange("p (h c) -> p h c", c=512)
                            nc.gpsimd.affine_select(
                                out=pt2v[:, :, c0:c0 + 128],
                                in_=pt2v[:, :, c0:c0 + 128],
                                pattern=[[0, 2], [1, 128]],
                                compare_op=GE, fill=0.0, base=0,
                                channel_multiplier=-1)
                        PT.append(pt2)
                    oA = psV.tile([65, 512], F32, tag="oA")
                    oB = psV.tile([65, 512], F32, tag="oB")
                    for kb in range(nkb):
                        first, last = kb == 0, kb == nkb - 1
                        nc.tensor.matmul(oA, v2_sb[kb][:, 65 * hA:65 * hA + 65],
                                         PT[kb][:, 0:512], start=first, stop=last)
                        nc.tensor.matmul(oB, v2_sb[kb][:, 65 * hB:65 * hB + 65],
                                         PT[kb][:, 512:1024], start=first, stop=last)
                    # normalize: r = 1/l (row 64), broadcast rows via DMA
                    for o_ps, half in ((oA, 0), (oB, 1)):
                        ltmp = nrm.tile([1, 512], F32, tag="ltmp")
                        nc.vector.tensor_copy(out=ltmp, in_=o_ps[64:65, :])
                        rrow = nrm.tile([1, 512], F32, tag="rrow")
                        nc.vector.reciprocal_approx_fast(out=rrow, in_=ltmp)
                        rbc = nrm.tile([64, 512], F32, tag="rbc")
                        nc.gpsimd.partition_broadcast(rbc, rrow)
                        nc.vector.tensor_mul(
                            out=attnT[p][64 * half:64 * (half + 1), qsl],
                            in0=o_ps[0:64, :], in1=rbc)

        # ---- Phase E: output projection ----
        with tc.tile_pool(name="psE", bufs=4, space="PSUM") as psE, \
             tc.tile_pool(name="outp", bufs=3) as op:
            for st in range(16):
                ob = op.tile([128, 1024], F32, tag="ob")
                for oc in range(2):
                    pe = psE.tile([128, 512], F32, tag="pe")
                    nc.tensor.matmul(
                        pe,
                        attnT[0][:, 128 * st:128 * (st + 1)],
                        woT_sb[0][:, 512 * oc:512 * (oc + 1)],
                        start=True, stop=False)
                    nc.tensor.matmul(
                        pe,
                        attnT[1][:, 128 * st:128 * (st + 1)],
                        woT_sb[1][:, 512 * oc:512 * (oc + 1)],
                        start=False, stop=True)
                    if oc == 0:
                        nc.vector.tensor_copy(out=ob[:, 0:512], in_=pe)
                    else:
                        nc.scalar.copy(out=ob[:, 512:1024], in_=pe)
                nc.sync.dma_start(out=out[128 * st:128 * (st + 1), 0:512], in_=ob[:, 0:512])
                nc.sync.dma_start(out=out[128 * st:128 * (st + 1), 512:1024], in_=ob[:, 512:1024])


def _build_program():
    nc = bacc.Bacc("TRN2", debug=False, enable_asserts=False,
                   target_bir_lowering=False, num_devices=NCORES)
    t_xT = nc.dram_tensor("xT", [D, S], F32, kind="ExternalInput")
    t_wqkT = nc.dram_tensor("wqkT", [D, 512], F32, kind="ExternalInput")
    t_wvT = nc.dram_tensor("wvT", [D, 256], F32, kind="ExternalInput")
    t_woT = nc.dram_tensor("woT", [256, D], F32, kind="ExternalInput")
    t_cos = nc.dram_tensor("cosd", [128, S], F32, kind="ExternalInput")
    t_ssin = nc.dram_tensor("ssin", [128, S], F32, kind="ExternalInput")
    t_perm = nc.dram_tensor("perm", [128, 128], F32, kind="ExternalInput")
    t_out = nc.dram_tensor("out", [S, D], F32, kind="ExternalOutput")
    with tile.TileContext(nc) as tc:
        _emit(tc, t_xT, t_wqkT, t_wvT, t_woT, t_cos, t_ssin, t_perm, t_out)
    nc.compile()
    return nc


def _rope_tables():
    # [128, S] tables for a 2-head stacked [d, s] block (pattern repeats per 64)
    i = np.arange(0, DK, 2, dtype=np.float64) / DK
    inv_freq = ROPE_THETA ** i                       # [32]
    ang = np.arange(S, dtype=np.float64)[None, :] / inv_freq[:, None]  # [32, S]
    cos64 = np.repeat(np.cos(ang), 2, axis=0)        # [64, S]
    sin = np.sin(ang)
    ssin64 = np.empty((DK, S), dtype=np.float64)
    ssin64[0::2] = -sin
    ssin64[1::2] = sin
    cos128 = np.tile(cos64, (2, 1)).astype(np.float32)
    ssin128 = np.tile(ssin64, (2, 1)).astype(np.float32)
    return np.ascontiguousarray(cos128), np.ascontiguousarray(ssin128)


def kernel(x, W_qkv, W_o):
    global _PROGRAM, LAST_RESULTS
    x = np.asarray(x, dtype=np.float32)
    W_qkv = np.asarray(W_qkv, dtype=np.float32)
    W_o = np.asarray(W_o, dtype=np.float32)

    if _PROGRAM is None:
        _PROGRAM = _build_program()
    nc = _PROGRAM

    cos128, ssin128 = _rope_tables()
    permM = np.zeros((128, 128), dtype=np.float32)
    idx = np.arange(128)
    permM[idx, idx ^ 1] = 1.0  # lhsT[K=d, M=d']: out[d'] = sum_d perm[d, d'] q[d] = q[d'^1]

    in_maps = []
    for c in range(NCORES):
        b, g = c // 4, c % 4
        rq = W_qkv[256 * g:256 * (g + 1)]
        rk = W_qkv[D + 256 * g:D + 256 * (g + 1)]
        rv = W_qkv[2 * D + 256 * g:2 * D + 256 * (g + 1)]
        in_maps.append({
            "xT": np.ascontiguousarray(x[b].T),
            "wqkT": np.ascontiguousarray(np.concatenate([rq, rk], 0).T),
            "wvT": np.ascontiguousarray(rv.T),
            "woT": np.ascontiguousarray(W_o[:, 256 * g:256 * (g + 1)].T),
            "cosd": cos128,
            "ssin": ssin128,
            "perm": permM,
        })

    res = run_bass_kernel_spmd(nc, in_maps, core_ids=list(range(NCORES)))
    LAST_RESULTS = res

    out = np.empty((B, S, D), dtype=np.float32)
    for b in range(B):
        acc = np.zeros((S, D), dtype=np.float64)
        for g in range(GROUPS):
            acc += res.results[4 * b + g]["out"]
        out[b] = acc.astype(np.float32)
    return out
